# revision 9
# baseline (speedup 1.0000x reference)
"""MoE routing kernel for Trainium2 (Bass/Tile), 8 NeuronCores.

Problem: DeepSeek-style MoE block: sigmoid router with group-limited top-k
(4 groups of 2 experts, top-2 groups -> top-4 experts), 8 routed experts
(SwiGLU, H=1024, I=512) evaluated densely with combine weights, plus a
shared expert, over N=8192 tokens.

Strategy:
  - Data-parallel: shard the 8192 tokens 8-ways (1024 tokens/core); all
    weights replicated per core. No collectives.
  - Per core, dense evaluation of all 8 experts + shared expert.
  - Layout: activations transposed once on-chip to xT [H, tok] via PE
    transposes. Gate/Up matmuls: lhsT = W chunks (native layout), rhs = xT
    -> g/u in [I, tok]. h = silu(g)*u on ACT+DVE. Down matmul: lhsT = h
    chunks, rhs = Wd (native layout) -> y in [tok, H] (output orientation).
  - Combine weights cw[token, e] are applied during the down-projection
    accumulation: acc[tok, H] += y_e * cw[:, e] with a single DVE
    scalar_tensor_tensor (cw is a per-partition scalar in this layout).
  - Router math in exact fp32 (PE fp32 matmul; group-score gaps go down to
    ~1.5e-5 so tf32 is not safe there). Expert matmuls in float32r (tf32,
    ~5e-4 rel err) which streams at full PE rate for moving dim >= 256.
  - float32r operands must be produced by rounding ops: expert weights are
    pre-rounded to tf32 on the host and DMA'd as f32r; xT is evacuated
    from the PE-transpose PSUM results twice (fp32 copy for the router,
    f32r copy for the experts); h is written as f32r by its DVE op.
"""

import numpy as np

import concourse.bass as bass
import concourse.bacc as bacc
import concourse.tile as tile
from concourse import mybir
from concourse.bass_utils import run_bass_kernel_spmd
from concourse.masks import make_identity

F32 = mybir.dt.float32
F32R = mybir.dt.float32r
AF = mybir.ActivationFunctionType
ALU = mybir.AluOpType
AX = mybir.AxisListType

B, T, H, I, E = 32, 256, 1024, 512, 8
N = B * T                     # 8192 tokens
NCORES = 8
NTOK = N // NCORES            # 1024 tokens per core
TOKT = NTOK // 128            # 8 token tiles per core
NB = 4                        # token blocks per core
TB = NTOK // NB               # 256 tokens per block
HK = H // 128                 # 8 contraction chunks over H
IK = I // 128                 # 4 chunks over I
SCALE = 2.5

TRACE = False
LAST_RESULT = None


def _build_kernel(sim_compat=False):
    nc = bacc.Bacc("TRN2", target_bir_lowering=False)

    x_d = nc.dram_tensor("x", [NTOK, H], F32, kind="ExternalInput")
    gw_d = nc.dram_tensor("gate_w", [E, H], F32, kind="ExternalInput")
    cb_d = nc.dram_tensor("correction_bias", [E], F32, kind="ExternalInput")
    # Expert weights are pre-rounded to tf32 on the host and declared f32r.
    wg_d = nc.dram_tensor("Wg", [E, H, I], F32R, kind="ExternalInput")
    wu_d = nc.dram_tensor("Wu", [E, H, I], F32R, kind="ExternalInput")
    wd_d = nc.dram_tensor("Wd", [E, I, H], F32R, kind="ExternalInput")
    wgs_d = nc.dram_tensor("Wg_s", [H, I], F32R, kind="ExternalInput")
    wus_d = nc.dram_tensor("Wu_s", [H, I], F32R, kind="ExternalInput")
    wds_d = nc.dram_tensor("Wd_s", [I, H], F32R, kind="ExternalInput")
    out_d = nc.dram_tensor("out", [NTOK, H], F32, kind="ExternalOutput")

    with tile.TileContext(nc) as tc:
        with (
            tc.tile_pool(name="const", bufs=1) as p_const,
            tc.tile_pool(name="xT", bufs=1) as p_xT,
            tc.tile_pool(name="work", bufs=6) as p_work,
            tc.tile_pool(name="wgu", bufs=6) as p_wgu,
            tc.tile_pool(name="wd", bufs=4) as p_wd,
            tc.tile_pool(name="acc", bufs=1) as p_acc,
            tc.tile_pool(name="small", bufs=4) as p_small,
            tc.tile_pool(name="cw", bufs=1) as p_cw,
            tc.tile_pool(name="psA", bufs=4, space="PSUM") as p_psA,
            tc.tile_pool(name="psY", bufs=2, space="PSUM") as p_psY,
        ):
            # ---------------- constants ----------------
            ident = p_const.tile([128, 128], F32, tag="ident")
            make_identity(nc, ident[:, :])

            # gate_w transposed: gwT[:, hk*8:(hk+1)*8] = gate_w[:, hk*128:+128].T
            gw_sb = p_const.tile([E, H], F32, tag="gwsb")
            nc.sync.dma_start(out=gw_sb[:, :], in_=gw_d.ap())
            gwT = p_const.tile([128, HK * E], F32, tag="gwT")
            for hk in range(HK):
                ps = p_psA.tile([128, 256], F32, tag="gu")
                nc.tensor.transpose(
                    ps[:, :E], gw_sb[:, hk * 128:(hk + 1) * 128], ident[:E, :E]
                )
                nc.scalar.activation(gwT[:, hk * E:(hk + 1) * E], ps[:, :E], AF.Copy)

            # correction bias broadcast to all partitions: biasb [128, E]
            biasb = p_const.tile([128, E], F32, tag="biasb")
            cb_bcast = bass.AP(
                tensor=cb_d.ap().tensor,
                offset=0,
                ap=[[0, 128], [1, E]],
            )
            nc.sync.dma_start(out=biasb[:, :], in_=cb_bcast)

            # ------------- x transpose + router, per block -------------
            # xTr [128, HK, NTOK] (f32r) is the expert-phase rhs.
            # Per block, a transient fp32 copy of the block's xT chunks feeds
            # the exact-fp32 router matmul.
            xTr = p_xT.tile([128, HK, NTOK], F32R, tag="xT")
            cw_all = p_cw.tile([128, TOKT, E], F32, tag="cw")

            for b in range(NB):
                t0 = b * TB
                xtb = []  # fp32 xT chunks for this block's router matmul
                for cc in range(TB // 128):
                    tt = (t0 // 128) + cc
                    x_in = p_work.tile([128, H], F32, tag="work")
                    nc.sync.dma_start(
                        out=x_in[:, :], in_=x_d.ap()[tt * 128:(tt + 1) * 128, :]
                    )
                    xb = p_work.tile([128, HK * 128], F32, tag="work")
                    for hk in range(HK):
                        ps = p_psA.tile([128, 256], F32, tag="gu")
                        nc.tensor.transpose(
                            ps[:, :128], x_in[:, hk * 128:(hk + 1) * 128], ident[:, :]
                        )
                        nc.vector.tensor_copy(
                            xTr[:, hk, tt * 128:(tt + 1) * 128], ps[:, :128]
                        )
                        nc.scalar.activation(
                            xb[:, hk * 128:(hk + 1) * 128], ps[:, :128], AF.Copy
                        )
                    xtb.append(xb)

                # logitsT [E, TB] = gate_w @ x[T].T  (exact fp32 matmul)
                ps_l = p_psA.tile([128, 256], F32, tag="gu")
                for hk in range(HK):
                    for cc in range(TB // 128):
                        nc.tensor.matmul(
                            ps_l[:E, cc * 128:(cc + 1) * 128],
                            gwT[:, hk * E:(hk + 1) * E],
                            xtb[cc][:, hk * 128:(hk + 1) * 128],
                            start=(hk == 0 and cc == 0),
                            stop=(hk == HK - 1 and cc == TB // 128 - 1),
                        )
                lT = p_small.tile([E, TB], F32, tag="lT")
                nc.scalar.activation(lT[:, :], ps_l[:E, :TB], AF.Copy)

                for cc in range(TB // 128):
                    c = (t0 // 128) + cc
                    ps_t = p_psA.tile([128, 256], F32, tag="gu")
                    nc.tensor.transpose(
                        ps_t[:, :E], lT[:, cc * 128:(cc + 1) * 128], ident[:E, :E]
                    )
                    scores = p_small.tile([128, E], F32, tag="scores")
                    nc.scalar.activation(scores[:, :], ps_t[:, :E], AF.Sigmoid)
                    scb = p_small.tile([128, E], F32, tag="scb")
                    nc.vector.tensor_tensor(scb[:, :], scores[:, :], biasb[:, :], ALU.add)
                    # group scores gs[g] = scb[2g] + scb[2g+1]
                    scb3 = scb.rearrange("p (g two) -> p g two", two=2)
                    gs = p_small.tile([128, 4], F32, tag="gs")
                    nc.vector.tensor_tensor(
                        gs[:, :],
                        scb3[:, :, 0:1].squeeze(),
                        scb3[:, :, 1:2].squeeze(),
                        ALU.add,
                    )
                    # pairwise "beats" with index tie-break (lower index wins)
                    beats = p_small.tile([128, 12], F32, tag="beats")
                    pairs = [(0, 1), (0, 2), (0, 3), (1, 2), (1, 3), (2, 3)]
                    for j, (a, bb) in enumerate(pairs):
                        nc.vector.tensor_tensor(
                            beats[:, j:j + 1], gs[:, a:a + 1], gs[:, bb:bb + 1], ALU.is_ge
                        )
                        nc.vector.tensor_tensor(
                            beats[:, 6 + j:7 + j], gs[:, bb:bb + 1], gs[:, a:a + 1], ALU.is_gt
                        )
                    # wins per group
                    wins = p_small.tile([128, 4], F32, tag="wins")
                    wcols = {
                        0: [0, 1, 2],       # ge01, ge02, ge03
                        1: [6, 3, 4],       # gt10, ge12, ge13
                        2: [7, 9, 5],       # gt20, gt21, ge23
                        3: [8, 10, 11],     # gt30, gt31, gt32
                    }
                    for g, (c0, c1, c2) in wcols.items():
                        nc.vector.tensor_tensor(
                            wins[:, g:g + 1], beats[:, c0:c0 + 1], beats[:, c1:c1 + 1], ALU.add
                        )
                        nc.vector.tensor_tensor(
                            wins[:, g:g + 1], wins[:, g:g + 1], beats[:, c2:c2 + 1], ALU.add
                        )
                    # selrep[2g] = selrep[2g+1] = (wins[g] >= 2)
                    selrep = p_small.tile([128, E], F32, tag="selrep")
                    for g in range(4):
                        for k in (0, 1):
                            nc.vector.tensor_scalar(
                                selrep[:, 2 * g + k:2 * g + k + 1],
                                wins[:, g:g + 1], 2.0, None, ALU.is_ge,
                            )
                    # masked scores, denom, cw
                    nc.vector.tensor_tensor(
                        selrep[:, :], selrep[:, :], scores[:, :], ALU.mult
                    )
                    denom = p_small.tile([128, 1], F32, tag="denom")
                    nc.vector.reduce_sum(denom[:, :], selrep[:, :], axis=AX.X)
                    nc.vector.tensor_scalar_add(denom[:, :], denom[:, :], 1e-20)
                    rcp = p_small.tile([128, 1], F32, tag="rcp")
                    nc.vector.reciprocal(rcp[:, :], denom[:, :])
                    nc.vector.tensor_scalar(
                        cw_all[:, c, :].squeeze(), selrep[:, :], rcp[:, :], float(SCALE),
                        ALU.mult, ALU.mult,
                    )

            # ---------------- experts ----------------
            acc = p_acc.tile([128, TOKT, H], F32, tag="acc")
            cw_flat = cw_all.rearrange("p t e -> p (t e)")

            def load_gu_half(dram, e, half):
                """[128, HK, 256] f32r tile: I-columns half*256..+256 of Wg/Wu."""
                t = p_wgu.tile([128, HK, 256], F32R, tag="wgu")
                if e < E:
                    src = dram.ap()[e, :, half * 256:(half + 1) * 256]
                else:
                    src = dram.ap()[:, half * 256:(half + 1) * 256]
                nc.sync.dma_start(
                    out=t[:, :, :], in_=src.rearrange("(hk p) i -> p hk i", p=128)
                )
                return t

            def load_wd_half(dram, e, half):
                """[128, 2, H] f32r tile: I-chunk rows half*256..+256 of Wd."""
                t = p_wd.tile([128, 2, H], F32R, tag="wd")
                if e < E:
                    src = dram.ap()[e, half * 256:(half + 1) * 256, :]
                else:
                    src = dram.ap()[half * 256:(half + 1) * 256, :]
                nc.sync.dma_start(
                    out=t[:, :, :], in_=src.rearrange("(kc p) h -> p kc h", p=128)
                )
                return t

            for e in range(E + 1):  # e == E is the shared expert
                shared = e == E
                wg_h = [load_gu_half(wgs_d if shared else wg_d, e, h2) for h2 in range(2)]
                wu_h = [load_gu_half(wus_d if shared else wu_d, e, h2) for h2 in range(2)]
                wd_h = [load_wd_half(wds_d if shared else wd_d, e, h2) for h2 in range(2)]

                for b in range(NB):
                    t0 = b * TB
                    # ---- up then gate: per I-chunk [128, TB] PSUM banks ----
                    u_sb = p_work.tile([128, I // 128 * TB], F32, tag="work")
                    sg_sb = p_work.tile([128, I // 128 * TB], F32, tag="work")
                    silu_f = AF.Sigmoid if sim_compat else AF.Silu
                    for dst, w_h, func in ((u_sb, wu_h, AF.Copy), (sg_sb, wg_h, silu_f)):
                        for ik in range(IK):
                            ps = p_psA.tile([128, 256], F32, tag="gu")
                            for hk in range(HK):
                                nc.tensor.matmul(
                                    ps[:, :],
                                    w_h[ik // 2][:, hk, (ik % 2) * 128:(ik % 2 + 1) * 128],
                                    xTr[:, hk, t0:t0 + TB],
                                    start=(hk == 0),
                                    stop=(hk == HK - 1),
                                )
                            nc.scalar.activation(
                                dst[:, ik * TB:(ik + 1) * TB], ps[:, :], func
                            )
                            if sim_compat and func == AF.Sigmoid:
                                # silu(g) = g * sigmoid(g); CoreSim lacks Silu
                                nc.vector.tensor_tensor(
                                    dst[:, ik * TB:(ik + 1) * TB],
                                    dst[:, ik * TB:(ik + 1) * TB], ps[:, :], ALU.mult,
                                )
                    # h = silu(g) * u, rounded to f32r by the DVE op
                    h_sb = p_work.tile([128, I // 128 * TB], F32R, tag="work")
                    nc.vector.tensor_tensor(h_sb[:, :], sg_sb[:, :], u_sb[:, :], ALU.mult)

                    # ---- down: y[tok, H] per 128-token tile, fold into acc ----
                    for m in range(TB // 128):
                        tt = (t0 // 128) + m
                        y_ps = p_psY.tile([128, H], F32, tag="y")
                        for ik in range(IK):
                            lhsT = h_sb[:, ik * TB + m * 128: ik * TB + (m + 1) * 128]
                            for nh in range(2):
                                nc.tensor.matmul(
                                    y_ps[:, nh * 512:(nh + 1) * 512],
                                    lhsT,
                                    wd_h[ik // 2][:, ik % 2, nh * 512:(nh + 1) * 512],
                                    start=(ik == 0),
                                    stop=(ik == IK - 1),
                                )
                        acc_sl = acc[:, tt, :].squeeze()
                        cw_col = None if shared else cw_flat[:, tt * E + e:tt * E + e + 1]
                        if shared:
                            nc.vector.tensor_tensor(acc_sl, acc_sl, y_ps[:, :], ALU.add)
                        elif e == 0:
                            nc.vector.tensor_scalar(
                                acc_sl, y_ps[:, :], cw_col, None, ALU.mult,
                            )
                        else:
                            nc.vector.scalar_tensor_tensor(
                                acc_sl, y_ps[:, :], cw_col, acc_sl, ALU.mult, ALU.add,
                            )

            # ---------------- store ----------------
            for tt in range(TOKT):
                nc.sync.dma_start(
                    out=out_d.ap()[tt * 128:(tt + 1) * 128, :],
                    in_=acc[:, tt, :].squeeze(),
                )

    if not nc.is_finalized():
        nc.finalize()
    return nc


_NC_CACHE = None


def _get_nc():
    global _NC_CACHE
    if _NC_CACHE is None:
        _NC_CACHE = _build_kernel()
    return _NC_CACHE


def _tf32(x):
    """Round fp32 ndarray to tf32 (10-bit mantissa, round-to-nearest-even)."""
    u = np.ascontiguousarray(x).view(np.uint32)
    r = (u + np.uint32(0x0FFF) + ((u >> np.uint32(13)) & np.uint32(1))) & np.uint32(
        0xFFFFE000
    )
    return r.view(np.float32)


def kernel(**inputs):
    global LAST_RESULT
    hs = np.ascontiguousarray(np.asarray(inputs["hidden_states"], dtype=np.float32))
    x = hs.reshape(N, H)

    def f32(k):
        return np.ascontiguousarray(np.asarray(inputs[k], np.float32))

    shared_map = {
        "gate_w": f32("gate_w"),
        "correction_bias": f32("correction_bias"),
        "Wg": _tf32(f32("Wg")),
        "Wu": _tf32(f32("Wu")),
        "Wd": _tf32(f32("Wd")),
        "Wg_s": _tf32(f32("Wg_s")),
        "Wu_s": _tf32(f32("Wu_s")),
        "Wd_s": _tf32(f32("Wd_s")),
    }
    in_maps = []
    for c in range(NCORES):
        m = dict(shared_map)
        m["x"] = np.ascontiguousarray(x[c * NTOK:(c + 1) * NTOK])
        in_maps.append(m)

    nc = _get_nc()
    res = run_bass_kernel_spmd(
        nc, in_maps, core_ids=list(range(NCORES)), trace=TRACE
    )
    LAST_RESULT = res
    out = np.concatenate([res.results[c]["out"] for c in range(NCORES)], axis=0)
    return out.reshape(B, T, H).astype(np.float32)
